# revision 1
# baseline (speedup 1.0000x reference)
"""Trainium2 Bass kernel for batched GNN message passing.

Computes, for each batch b:
    ax[b] = segment_sum(x[b][cols] * vals[:, None], rows, num_segments=N_OUT)
    out[b] = ax[b] @ weight + bias

Strategy (8 NeuronCores):
  * The two batches share one graph, so x is interleaved host-side into
    xi[n] = concat(x[0,n], x[1,n]) -> [N_IN, 2*IN_C]; one gathered row
    serves both batches (halves descriptor count, shares the selection
    matrix and the PE weight load between batches).
  * Output rows are split into 128-row blocks. Blocks are dealt to the 8
    cores sorted by edge count (rounds of 8 similar-sized blocks) so every
    core runs an identical program: NB block-slots, slot s processing
    TPBL[s] + TPBH[s] tiles of 128 edges (dma_gather indices are int16,
    so edges are split into col < 32768 gathered from xi[0:] and
    col >= 32768 gathered from xi[32768:]).
  * Per 128-edge tile: dma_gather of 128 rows (2KB each) from HBM, build
    the scaled selection matrix S^T[e, r] = vals[e] * (rloc[e] == r) with
    one fused DVE op, then PE matmul psum[128 rows, 2*IN_C] += S^T.T @ G
    accumulated over the slot's tiles (the segment sum).
  * Per slot epilogue: transpose the segment sum (PE), project with
    weight (PE, K=256 in two 128-chunks), add bias (DVE) and DMA out.
"""

import math

import numpy as np

# ---------------------------------------------------------------- problem dims
B = 2
N_IN = 50000
N_OUT = 12500
NNZ = 500000
IN_C = 256
OUT_C = 256
N_CORES = 8
PB = 128  # rows per output block == partition count
H16 = 32768  # int16 index limit for dma_gather

import os as _os

GCH = int(_os.environ.get("K_GCH", "8"))  # 128-edge tiles per dma_gather call
GDT = _os.environ.get("K_GDT", "f32r")  # "f32" | "f32r" (tf32-like) | "f16"
# SWDGE queues: gather descriptor-gen spread over NSWQ Q7 core pairs
NSWQ = int(_os.environ.get("K_NSWQ", "4"))

_CACHE = {}
LAST_RESULTS = None


# ---------------------------------------------------------------- host planning
def _plan(rows, cols):
    """Pack output rows into (core, slot) bins of <=128 rows, balancing the
    low/high edge loads so every slot's tile counts are tight and uniform.
    Slot s everywhere holds TPBL[s] low tiles + TPBH[s] high tiles."""
    Lr = np.bincount(rows[cols < H16], minlength=N_OUT)
    Hr = np.bincount(rows[cols >= H16], minlength=N_OUT)
    NB = -(-N_OUT // (PB * N_CORES))
    nbins = N_CORES * NB
    tL = max(Lr.sum() / nbins, 1.0)
    tH = max(Hr.sum() / nbins, 1.0)

    order = np.argsort(-(Lr + Hr), kind="stable")
    binL = np.zeros(nbins)
    binH = np.zeros(nbins)
    binN = np.zeros(nbins, dtype=np.int64)
    bin_rows = [[] for _ in range(nbins)]
    for r in order:
        score = np.maximum((binL + Lr[r]) / tL, (binH + Hr[r]) / tH)
        score[binN >= PB] = np.inf
        b = int(score.argmin())
        binL[b] += Lr[r]
        binH[b] += Hr[r]
        binN[b] += 1
        bin_rows[b].append(int(r))

    # group bins into slots by (L-quantum, H) so per-slot maxima stay tight
    q = np.lexsort((-binH, -(-(-binL.astype(np.int64) // PB))))
    rowsets = [[None] * NB for _ in range(N_CORES)]
    TPBL, TPBH = [], []
    for s in range(NB):
        grp = q[s * N_CORES : (s + 1) * N_CORES]
        tl = int(-(-int(binL[grp].max()) // PB))
        th = int(-(-int(binH[grp].max()) // PB))
        if tl + th == 0:
            tl = 1
        TPBL.append(tl)
        TPBH.append(th)
        for c, b in enumerate(grp):
            rowsets[c][s] = np.array(sorted(bin_rows[b]), dtype=np.int64)
    return NB, TPBL, TPBH, rowsets


def _wrap16(flat):
    """int16 index stream -> dma_gather layout: idx i at partition i%16,
    col i//16, replicated across the 8 gpsimd core groups."""
    n = len(flat)
    assert n % 16 == 0
    w = np.ascontiguousarray(flat.reshape(n // 16, 16).T.astype(np.int16))
    return np.ascontiguousarray(np.tile(w, (8, 1)))


def _pack_core(c, plan, rows, cols, vals, bias, order_r, bnd_r):
    """Per-core arrays: rowsT/valsT [128, NT] (col j = tile j, partition p
    = edge j*128+p), wrapped int16 gather index streams, bias."""
    NB, TPBL, TPBH, rowsets = plan
    NT = sum(TPBL) + sum(TPBH)
    rloc_flat = np.zeros(NT * PB, dtype=np.float32)
    vals_flat = np.zeros(NT * PB, dtype=np.float32)
    lowE, highE = [], []
    bias_c = np.zeros((NB * PB, OUT_C), dtype=np.float32)
    pos = 0
    for s in range(NB):
        rowlist = rowsets[c][s]
        nr = len(rowlist)
        if nr:
            per_row = [order_r[bnd_r[r] : bnd_r[r + 1]] for r in rowlist]
            eids = (np.concatenate(per_row) if per_row
                    else np.empty(0, np.int64))
            rloc = np.repeat(np.arange(nr), [len(e) for e in per_row])
            m = cols[eids] < H16
            lo, lo_rl = eids[m], rloc[m]
            hi, hi_rl = eids[~m], rloc[~m]
            # column-sorted gather streams sweep HBM monotonically
            ol = np.argsort(cols[lo], kind="stable")
            lo, lo_rl = lo[ol], lo_rl[ol]
            oh = np.argsort(cols[hi], kind="stable")
            hi, hi_rl = hi[oh], hi_rl[oh]
            bias_c[s * PB : s * PB + nr] = bias[rowlist]
        else:
            lo = hi = np.empty(0, dtype=np.int64)
            lo_rl = hi_rl = np.empty(0, dtype=np.int64)
        for lst, rl, tpb, base, acc in (
            (lo, lo_rl, TPBL[s], 0, lowE),
            (hi, hi_rl, TPBH[s], H16, highE),
        ):
            k = tpb * PB
            if k == 0:
                assert len(lst) == 0
                continue
            ne = len(lst)
            assert ne <= k, (ne, k)
            rloc_flat[pos : pos + ne] = rl
            vals_flat[pos : pos + ne] = vals[lst]
            cc = np.zeros(k, dtype=np.int64)
            cc[:ne] = cols[lst] - base
            acc.append(cc)
            pos += k
    assert pos == NT * PB

    rowsT = np.ascontiguousarray(rloc_flat.reshape(NT, PB).T)
    valsT = np.ascontiguousarray(vals_flat.reshape(NT, PB).T)
    idxLW = _wrap16(np.concatenate(lowE) if lowE else np.empty(0, np.int64))
    idxHW = _wrap16(np.concatenate(highE) if highE else np.empty(0, np.int64))
    return rowsT, valsT, idxLW, idxHW, bias_c


# ---------------------------------------------------------------- device build
def _build(NB, TPBL, TPBH, n_in, nbatch, in_c, out_c, gch, reps=1, gdt=None):
    import concourse.bacc as bacc
    import concourse.mybir as mybir
    from concourse.tile import TileContext

    f32 = mybir.dt.float32
    i16 = mybir.dt.int16
    gdt = {"f32": f32, "f32r": mybir.dt.float32r,
           "f16": mybir.dt.float16}[gdt or GDT]
    C2 = nbatch * in_c  # gathered row width (both batches)
    NL, NH = sum(TPBL), sum(TPBH)
    NT = NL + NH
    NK = in_c // PB  # K-chunks in the weight projection

    nc = bacc.Bacc("TRN2", target_bir_lowering=False, debug=False,
                   num_devices=N_CORES, num_swdge_queues=NSWQ)

    xi_d = nc.dram_tensor("xi", [n_in, C2], gdt, kind="ExternalInput")
    idxl_d = nc.dram_tensor("idxLW", [PB, max(NL * 8, 8)], i16, kind="ExternalInput")
    idxh_d = nc.dram_tensor("idxHW", [PB, max(NH * 8, 8)], i16, kind="ExternalInput")
    rows_d = nc.dram_tensor("rowsT", [PB, NT], f32, kind="ExternalInput")
    vals_d = nc.dram_tensor("valsT", [PB, NT], f32, kind="ExternalInput")
    bias_d = nc.dram_tensor("biasC", [NB * PB, out_c], f32, kind="ExternalInput")
    w_d = nc.dram_tensor("wT", [PB, NK * out_c], f32, kind="ExternalInput")
    iota_d = nc.dram_tensor("iota", [PB, PB], f32, kind="ExternalInput")
    ident_d = nc.dram_tensor("ident", [PB, PB], f32, kind="ExternalInput")
    out_d = nc.dram_tensor("out", [nbatch, NB * PB, out_c], f32,
                           kind="ExternalOutput")

    # processing order: per slot, its low tiles then its high tiles.
    # each entry: (slot, first, last, stream ('l'/'h'), stream tile index)
    sched = []
    nl = nh = 0
    for s in range(NB):
        tpb = TPBL[s] + TPBH[s]
        t = 0
        for _ in range(TPBL[s]):
            sched.append((s, t == 0, t == tpb - 1, "l", nl))
            nl += 1
            t += 1
        for _ in range(TPBH[s]):
            sched.append((s, t == 0, t == tpb - 1, "h", nh))
            nh += 1
            t += 1

    def body(nc, tc, pools):
        (cpool, gl_pool, gh_pool, spool, segpool, trpool, opool, bpool,
         papool, ptpool, popool) = pools
        iota_sb = cpool.tile([PB, PB], f32, tag="iota")
        ident_sb = cpool.tile([PB, PB], f32, tag="ident")
        w_sb = cpool.tile([PB, NK * out_c], f32, tag="w")
        idxl_sb = cpool.tile([PB, max(NL * 8, 8)], i16, tag="idxl")
        idxh_sb = cpool.tile([PB, max(NH * 8, 8)], i16, tag="idxh")
        rows_sb = cpool.tile([PB, NT], f32, tag="rows")
        vals_sb = cpool.tile([PB, NT], f32, tag="vals")
        nc.sync.dma_start(out=iota_sb[:], in_=iota_d[:])
        nc.sync.dma_start(out=ident_sb[:], in_=ident_d[:])
        nc.sync.dma_start(out=w_sb[:], in_=w_d[:])
        nc.sync.dma_start(out=idxl_sb[:], in_=idxl_d[:])
        nc.sync.dma_start(out=idxh_sb[:], in_=idxh_d[:])
        nc.sync.dma_start(out=rows_sb[:], in_=rows_d[:])
        nc.sync.dma_start(out=vals_sb[:], in_=vals_d[:])

        qctr = [0]

        def gather(pool, tag, idx_sb, src_ap, t0, nstream):
            n = min(gch, nstream - t0)
            g = pool.tile([PB, gch * C2], gdt, tag=tag)
            nc.gpsimd.dma_gather(
                out_ap=g[:, : n * C2].rearrange("p (t e) -> p t e", e=C2),
                in_ap=src_ap,
                idxs_ap=idx_sb[:, t0 * 8 : (t0 + n) * 8],
                num_idxs=n * PB,
                num_idxs_reg=n * PB,
                elem_size=C2,
                queue_num=qctr[0] % NSWQ,
            )
            qctr[0] += 1
            return g

        gl = gh = None
        pacc = None
        bias_sb = None
        cur_s = -1
        for j in range(NT):
            s, first, last, stream, st = sched[j]
            if stream == "l":
                if st % gch == 0:
                    gl = gather(gl_pool, "gl", idxl_sb, xi_d[:], st, NL)
                g, off = gl, st % gch
            else:
                if st % gch == 0:
                    gh = gather(gh_pool, "gh", idxh_sb, xi_d[H16:, :], st, NH)
                g, off = gh, st % gch
            if first:
                cur_s = s
                bias_sb = bpool.tile([PB, out_c], f32, tag="bias")
                nc.sync.dma_start(
                    out=bias_sb[:], in_=bias_d[s * PB : (s + 1) * PB, :]
                )
                pacc = papool.tile([PB, C2], f32)

            s_t = spool.tile([PB, PB], gdt, tag="s")
            nc.vector.tensor_scalar(
                out=s_t[:],
                in0=iota_sb[:],
                scalar1=rows_sb[:, j : j + 1],
                scalar2=vals_sb[:, j : j + 1],
                op0=mybir.AluOpType.is_equal,
                op1=mybir.AluOpType.mult,
            )
            nc.tensor.matmul(
                out=pacc[:],
                lhsT=s_t[:],
                rhs=g[:, off * C2 : (off + 1) * C2],
                start=first,
                stop=last,
            )

            if last:
                seg = segpool.tile([PB, C2], f32, tag="seg")
                nc.scalar.copy(out=seg[:], in_=pacc[:])
                for b in range(nbatch):
                    trs = []
                    for k in range(NK):
                        ptr = ptpool.tile([PB, PB], f32)
                        nc.tensor.transpose(
                            out=ptr[:],
                            in_=seg[:, b * in_c + k * PB : b * in_c + (k + 1) * PB],
                            identity=ident_sb[:],
                        )
                        trk = trpool.tile([PB, PB], f32, tag="tr")
                        nc.scalar.copy(out=trk[:], in_=ptr[:])
                        trs.append(trk)
                    po = popool.tile([PB, out_c], f32)
                    for k in range(NK):
                        nc.tensor.matmul(
                            out=po[:],
                            lhsT=trs[k][:],
                            rhs=w_sb[:, k * out_c : (k + 1) * out_c],
                            start=(k == 0),
                            stop=(k == NK - 1),
                        )
                    osb = opool.tile([PB, out_c], f32, tag="o")
                    nc.vector.tensor_tensor(
                        out=osb[:], in0=po[:], in1=bias_sb[:],
                        op=mybir.AluOpType.add,
                    )
                    nc.sync.dma_start(
                        out=out_d[b, cur_s * PB : (cur_s + 1) * PB, :],
                        in_=osb[:],
                    )

    with TileContext(nc) as tc:
        with (
            tc.tile_pool(name="const", bufs=1) as cpool,
            tc.tile_pool(name="gl", bufs=3) as gl_pool,
            tc.tile_pool(name="gh", bufs=3) as gh_pool,
            tc.tile_pool(name="s", bufs=6) as spool,
            tc.tile_pool(name="seg", bufs=2) as segpool,
            tc.tile_pool(name="tr", bufs=4) as trpool,
            tc.tile_pool(name="o", bufs=4) as opool,
            tc.tile_pool(name="bias", bufs=2) as bpool,
            tc.tile_pool(name="pacc", bufs=2, space="PSUM") as papool,
            tc.tile_pool(name="ptr", bufs=2, space="PSUM") as ptpool,
            tc.tile_pool(name="pout", bufs=2, space="PSUM") as popool,
        ):
            pools = (cpool, gl_pool, gh_pool, spool, segpool, trpool, opool,
                     bpool, papool, ptpool, popool)
            if reps == 1:
                body(nc, tc, pools)
            else:
                with tc.For_i(0, reps, 1):
                    body(nc, tc, pools)

    nc.compile()
    return nc


def _host_arrays(x, weight):
    xi = np.ascontiguousarray(np.concatenate([x[b] for b in range(B)], axis=1))
    NK = IN_C // PB
    wT = np.ascontiguousarray(
        np.concatenate([weight[k * PB : (k + 1) * PB] for k in range(NK)], axis=1)
    )
    iota2d = np.ascontiguousarray(np.tile(np.arange(PB, dtype=np.float32), (PB, 1)))
    ident = np.eye(PB, dtype=np.float32)
    return xi, wT, iota2d, ident


def _in_maps(rows, cols, vals, weight, bias, x, plan):
    NB, TPBL, TPBH, rowsets = plan
    xi, wT, iota2d, ident = _host_arrays(x, weight)
    if GDT == "f16":
        xi = xi.astype(np.float16)
    order_r = np.argsort(rows, kind="stable")
    bnd_r = np.searchsorted(rows[order_r], np.arange(N_OUT + 1))
    maps = []
    for c in range(N_CORES):
        rowsT, valsT, idxLW, idxHW, bias_c = _pack_core(
            c, plan, rows, cols, vals, bias, order_r, bnd_r
        )
        if idxLW.size == 0:
            idxLW = np.zeros((PB, 8), np.int16)
        if idxHW.size == 0:
            idxHW = np.zeros((PB, 8), np.int16)
        maps.append(
            {
                "xi": xi,
                "idxLW": idxLW,
                "idxHW": idxHW,
                "rowsT": rowsT,
                "valsT": valsT,
                "biasC": bias_c,
                "wT": wT,
                "iota": iota2d,
                "ident": ident,
            }
        )
    return maps


def time_hw(inputs, reps=(1, 129), trials=4):
    """HW ns/iter via wall-clock delta between For_i repeat-count variants.
    Upload + dispatch costs are identical across variants and cancel."""
    import time as _time

    from concourse.bass_utils import run_bass_kernel_spmd

    rows = np.asarray(inputs["rows"], dtype=np.int64)
    cols = np.asarray(inputs["cols"], dtype=np.int64)
    vals = np.asarray(inputs["vals"], dtype=np.float32)
    x = np.asarray(inputs["x"], dtype=np.float32)
    weight = np.asarray(inputs["weight"], dtype=np.float32)
    bias = np.asarray(inputs["bias"], dtype=np.float32)

    plan = _plan(rows, cols)
    NB, TPBL, TPBH, rowsets = plan
    maps = _in_maps(rows, cols, vals, weight, bias, x, plan)

    best = {}
    for r in reps:
        nc = _build(NB, TPBL, TPBH, N_IN, B, IN_C, OUT_C, GCH, reps=r)
        run_bass_kernel_spmd(nc, maps, core_ids=list(range(N_CORES)))  # warm
        ts = []
        for _ in range(trials):
            t0 = _time.perf_counter()
            run_bass_kernel_spmd(nc, maps, core_ids=list(range(N_CORES)))
            ts.append(_time.perf_counter() - t0)
        best[r] = min(ts)
        print(f"reps={r}: calls {[f'{t*1e3:.1f}ms' for t in ts]}")
    r1, r2 = min(reps), max(reps)
    ns = (best[r2] - best[r1]) / (r2 - r1) * 1e9
    return ns


# ---------------------------------------------------------------- entry point
def kernel(x, rows, cols, vals, weight, bias):
    global LAST_RESULTS
    from concourse.bass_utils import run_bass_kernel_spmd

    x = np.asarray(x, dtype=np.float32)
    rows = np.asarray(rows, dtype=np.int64)
    cols = np.asarray(cols, dtype=np.int64)
    vals = np.asarray(vals, dtype=np.float32)
    weight = np.asarray(weight, dtype=np.float32)
    bias = np.asarray(bias, dtype=np.float32)

    plan = _plan(rows, cols)
    NB, TPBL, TPBH, rowsets = plan

    key = (NB, tuple(TPBL), tuple(TPBH), GCH)
    if key not in _CACHE:
        _CACHE.clear()
        _CACHE[key] = _build(NB, TPBL, TPBH, N_IN, B, IN_C, OUT_C, GCH)
    nc = _CACHE[key]

    maps = _in_maps(rows, cols, vals, weight, bias, x, plan)
    res = run_bass_kernel_spmd(nc, maps, core_ids=list(range(N_CORES)))
    LAST_RESULTS = res

    out = np.empty((B, N_OUT, OUT_C), dtype=np.float32)
    for c in range(N_CORES):
        oc = res.results[c]["out"]
        for s in range(NB):
            rowlist = rowsets[c][s]
            if rowlist is None or len(rowlist) == 0:
                continue
            out[:, rowlist, :] = oc[:, s * PB : s * PB + len(rowlist), :]
    return out



# revision 5
# speedup vs baseline: 1.1351x; 1.1351x over previous
"""Trainium2 Bass kernel for batched GNN message passing.

Computes, for each batch b:
    ax[b] = segment_sum(x[b][cols] * vals[:, None], rows, num_segments=N_OUT)
    out[b] = ax[b] @ weight + bias

Strategy (8 NeuronCores):
  * The two batches share one graph, so x is interleaved host-side into
    xi[n] = concat(x[0,n], x[1,n]) -> [N_IN, 2*IN_C]; one gathered row
    serves both batches (halves descriptor count, shares the selection
    matrix and the PE weight load between batches).
  * Output rows are split into 128-row blocks. Blocks are dealt to the 8
    cores sorted by edge count (rounds of 8 similar-sized blocks) so every
    core runs an identical program: NB block-slots, slot s processing
    TPBL[s] + TPBH[s] tiles of 128 edges (dma_gather indices are int16,
    so edges are split into col < 32768 gathered from xi[0:] and
    col >= 32768 gathered from xi[32768:]).
  * Per 128-edge tile: dma_gather of 128 rows (2KB each) from HBM, build
    the scaled selection matrix S^T[e, r] = vals[e] * (rloc[e] == r) with
    one fused DVE op, then PE matmul psum[128 rows, 2*IN_C] += S^T.T @ G
    accumulated over the slot's tiles (the segment sum).
  * Per slot epilogue: transpose the segment sum (PE), project with
    weight (PE, K=256 in two 128-chunks), add bias (DVE) and DMA out.
"""

import math

import numpy as np

# ---------------------------------------------------------------- problem dims
B = 2
N_IN = 50000
N_OUT = 12500
NNZ = 500000
IN_C = 256
OUT_C = 256
N_CORES = 8
PB = 128  # rows per output block == partition count
H16 = 32768  # int16 index limit for dma_gather

import os as _os

GCH = int(_os.environ.get("K_GCH", "8"))  # 128-edge tiles per dma_gather call
# gather dtype: "f32" | "f32r" (tf32-like) | "f16" | "f8" (e3m4)
GDT = _os.environ.get("K_GDT", "f32r")
# SWDGE queues: gather descriptor-gen spread over NSWQ Q7 core pairs
NSWQ = int(_os.environ.get("K_NSWQ", "4"))

_CACHE = {}
LAST_RESULTS = None


# ---------------------------------------------------------------- host planning
def _plan(rows, cols):
    """Pack output rows into (core, slot) bins of <=128 rows, balancing the
    low/high edge loads so every slot's tile counts are tight and uniform.
    Slot s everywhere holds TPBL[s] low tiles + TPBH[s] high tiles."""
    Lr = np.bincount(rows[cols < H16], minlength=N_OUT)
    Hr = np.bincount(rows[cols >= H16], minlength=N_OUT)
    NB = -(-N_OUT // (PB * N_CORES))
    nbins = N_CORES * NB
    tL = max(Lr.sum() / nbins, 1.0)
    tH = max(Hr.sum() / nbins, 1.0)

    order = np.argsort(-(Lr + Hr), kind="stable")
    binL = np.zeros(nbins)
    binH = np.zeros(nbins)
    binN = np.zeros(nbins, dtype=np.int64)
    bin_rows = [[] for _ in range(nbins)]
    for r in order:
        score = np.maximum((binL + Lr[r]) / tL, (binH + Hr[r]) / tH)
        score[binN >= PB] = np.inf
        b = int(score.argmin())
        binL[b] += Lr[r]
        binH[b] += Hr[r]
        binN[b] += 1
        bin_rows[b].append(int(r))

    # group bins into slots by (L-quantum, H) so per-slot maxima stay tight
    q = np.lexsort((-binH, -(-(-binL.astype(np.int64) // PB))))
    rowsets = [[None] * NB for _ in range(N_CORES)]
    TPBL, TPBH = [], []
    for s in range(NB):
        grp = q[s * N_CORES : (s + 1) * N_CORES]
        tl = int(-(-int(binL[grp].max()) // PB))
        th = int(-(-int(binH[grp].max()) // PB))
        if tl + th == 0:
            tl = 1
        TPBL.append(tl)
        TPBH.append(th)
        for c, b in enumerate(grp):
            rowsets[c][s] = np.array(sorted(bin_rows[b]), dtype=np.int64)
    return NB, TPBL, TPBH, rowsets


def _wrap16(flat):
    """int16 index stream -> dma_gather layout: idx i at partition i%16,
    col i//16, replicated across the 8 gpsimd core groups."""
    n = len(flat)
    assert n % 16 == 0
    w = np.ascontiguousarray(flat.reshape(n // 16, 16).T.astype(np.int16))
    return np.ascontiguousarray(np.tile(w, (8, 1)))


def _pack_core(c, plan, rows, cols, vals, bias, order_r, bnd_r):
    """Per-core arrays: rowsT/valsT [128, NT] (col j = tile j, partition p
    = edge j*128+p), wrapped int16 gather index streams, bias."""
    NB, TPBL, TPBH, rowsets = plan
    NT = sum(TPBL) + sum(TPBH)
    rloc_flat = np.zeros(NT * PB, dtype=np.float32)
    vals_flat = np.zeros(NT * PB, dtype=np.float32)
    lowE, highE = [], []
    bias_c = np.zeros((NB * PB, OUT_C), dtype=np.float32)
    pos = 0
    for s in range(NB):
        rowlist = rowsets[c][s]
        nr = len(rowlist)
        if nr:
            per_row = [order_r[bnd_r[r] : bnd_r[r + 1]] for r in rowlist]
            eids = (np.concatenate(per_row) if per_row
                    else np.empty(0, np.int64))
            rloc = np.repeat(np.arange(nr), [len(e) for e in per_row])
            m = cols[eids] < H16
            lo, lo_rl = eids[m], rloc[m]
            hi, hi_rl = eids[~m], rloc[~m]
            # column-sorted gather streams sweep HBM monotonically
            ol = np.argsort(cols[lo], kind="stable")
            lo, lo_rl = lo[ol], lo_rl[ol]
            oh = np.argsort(cols[hi], kind="stable")
            hi, hi_rl = hi[oh], hi_rl[oh]
            bias_c[s * PB : s * PB + nr] = bias[rowlist]
        else:
            lo = hi = np.empty(0, dtype=np.int64)
            lo_rl = hi_rl = np.empty(0, dtype=np.int64)
        for lst, rl, tpb, base, acc in (
            (lo, lo_rl, TPBL[s], 0, lowE),
            (hi, hi_rl, TPBH[s], H16, highE),
        ):
            k = tpb * PB
            if k == 0:
                assert len(lst) == 0
                continue
            ne = len(lst)
            assert ne <= k, (ne, k)
            rloc_flat[pos : pos + ne] = rl
            vals_flat[pos : pos + ne] = vals[lst]
            cc = np.zeros(k, dtype=np.int64)
            cc[:ne] = cols[lst] - base
            acc.append(cc)
            pos += k
    assert pos == NT * PB

    rowsT = np.ascontiguousarray(rloc_flat.reshape(NT, PB).T)
    valsT = np.ascontiguousarray(vals_flat.reshape(NT, PB).T)
    idxLW = _wrap16(np.concatenate(lowE) if lowE else np.empty(0, np.int64))
    idxHW = _wrap16(np.concatenate(highE) if highE else np.empty(0, np.int64))
    return rowsT, valsT, idxLW, idxHW, bias_c


# ---------------------------------------------------------------- device build
def _build(NB, TPBL, TPBH, n_in, nbatch, in_c, out_c, gch, reps=1, gdt=None):
    import concourse.bacc as bacc
    import concourse.mybir as mybir
    from concourse.tile import TileContext

    f32 = mybir.dt.float32
    i16 = mybir.dt.int16
    gdt = {"f32": f32, "f32r": mybir.dt.float32r,
           "f16": mybir.dt.float16, "f8": mybir.dt.float8e3}[gdt or GDT]
    C2 = nbatch * in_c  # gathered row width (both batches)
    NL, NH = sum(TPBL), sum(TPBH)
    NT = NL + NH
    NK = in_c // PB  # K-chunks in the weight projection

    nc = bacc.Bacc("TRN2", target_bir_lowering=False, debug=False,
                   num_devices=N_CORES, num_swdge_queues=NSWQ)

    xi_d = nc.dram_tensor("xi", [n_in, C2], gdt, kind="ExternalInput")
    idxl_d = nc.dram_tensor("idxLW", [PB, max(NL * 8, 8)], i16, kind="ExternalInput")
    idxh_d = nc.dram_tensor("idxHW", [PB, max(NH * 8, 8)], i16, kind="ExternalInput")
    rows_d = nc.dram_tensor("rowsT", [PB, NT], f32, kind="ExternalInput")
    vals_d = nc.dram_tensor("valsT", [PB, NT], f32, kind="ExternalInput")
    bias_d = nc.dram_tensor("biasC", [NB * PB, out_c], f32, kind="ExternalInput")
    w_d = nc.dram_tensor("wT", [PB, NK * out_c], f32, kind="ExternalInput")
    iota_d = nc.dram_tensor("iota", [PB, PB], f32, kind="ExternalInput")
    ident_d = nc.dram_tensor("ident", [PB, PB], f32, kind="ExternalInput")
    out_d = nc.dram_tensor("out", [nbatch, NB * PB, out_c], f32,
                           kind="ExternalOutput")

    # processing order: per slot, its low tiles then its high tiles.
    # each entry: (slot, first, last, stream ('l'/'h'), stream tile index)
    sched = []
    nl = nh = 0
    for s in range(NB):
        tpb = TPBL[s] + TPBH[s]
        t = 0
        for _ in range(TPBL[s]):
            sched.append((s, t == 0, t == tpb - 1, "l", nl))
            nl += 1
            t += 1
        for _ in range(TPBH[s]):
            sched.append((s, t == 0, t == tpb - 1, "h", nh))
            nh += 1
            t += 1

    def body(nc, tc, pools):
        (cpool, gl_pool, gh_pool, spool, segpool, trpool, opool, bpool,
         papool, ptpool, popool) = pools
        iota_sb = cpool.tile([PB, PB], f32, tag="iota")
        ident_sb = cpool.tile([PB, PB], f32, tag="ident")
        w_sb = cpool.tile([PB, NK * out_c], f32, tag="w")
        idxl_sb = cpool.tile([PB, max(NL * 8, 8)], i16, tag="idxl")
        idxh_sb = cpool.tile([PB, max(NH * 8, 8)], i16, tag="idxh")
        rows_sb = cpool.tile([PB, NT], f32, tag="rows")
        vals_sb = cpool.tile([PB, NT], f32, tag="vals")
        nc.sync.dma_start(out=iota_sb[:], in_=iota_d[:])
        nc.sync.dma_start(out=ident_sb[:], in_=ident_d[:])
        nc.sync.dma_start(out=w_sb[:], in_=w_d[:])
        nc.sync.dma_start(out=idxl_sb[:], in_=idxl_d[:])
        nc.sync.dma_start(out=idxh_sb[:], in_=idxh_d[:])
        nc.sync.dma_start(out=rows_sb[:], in_=rows_d[:])
        nc.sync.dma_start(out=vals_sb[:], in_=vals_d[:])

        qctr = [0]

        def gather(pool, tag, idx_sb, src_ap, t0, nstream):
            n = min(gch, nstream - t0)
            g = pool.tile([PB, gch * C2], gdt, tag=tag)
            nc.gpsimd.dma_gather(
                out_ap=g[:, : n * C2].rearrange("p (t e) -> p t e", e=C2),
                in_ap=src_ap,
                idxs_ap=idx_sb[:, t0 * 8 : (t0 + n) * 8],
                num_idxs=n * PB,
                num_idxs_reg=n * PB,
                elem_size=C2,
                queue_num=qctr[0] % NSWQ,
            )
            qctr[0] += 1
            return g

        gl = gh = None
        pacc = None
        bias_sb = None
        cur_s = -1
        for j in range(NT):
            s, first, last, stream, st = sched[j]
            if stream == "l":
                if st % gch == 0:
                    gl = gather(gl_pool, "gl", idxl_sb, xi_d[:], st, NL)
                g, off = gl, st % gch
            else:
                if st % gch == 0:
                    gh = gather(gh_pool, "gh", idxh_sb, xi_d[H16:, :], st, NH)
                g, off = gh, st % gch
            if first:
                cur_s = s
                bias_sb = bpool.tile([PB, out_c], f32, tag="bias")
                nc.sync.dma_start(
                    out=bias_sb[:], in_=bias_d[s * PB : (s + 1) * PB, :]
                )
                pacc = papool.tile([PB, C2], f32)

            s_t = spool.tile([PB, PB], gdt, tag="s")
            nc.vector.tensor_scalar(
                out=s_t[:],
                in0=iota_sb[:],
                scalar1=rows_sb[:, j : j + 1],
                scalar2=vals_sb[:, j : j + 1],
                op0=mybir.AluOpType.is_equal,
                op1=mybir.AluOpType.mult,
            )
            nc.tensor.matmul(
                out=pacc[:],
                lhsT=s_t[:],
                rhs=g[:, off * C2 : (off + 1) * C2],
                start=first,
                stop=last,
            )

            if last:
                seg = segpool.tile([PB, C2], f32, tag="seg")
                nc.scalar.copy(out=seg[:], in_=pacc[:])
                for b in range(nbatch):
                    trs = []
                    for k in range(NK):
                        ptr = ptpool.tile([PB, PB], f32)
                        nc.tensor.transpose(
                            out=ptr[:],
                            in_=seg[:, b * in_c + k * PB : b * in_c + (k + 1) * PB],
                            identity=ident_sb[:],
                        )
                        trk = trpool.tile([PB, PB], f32, tag="tr")
                        nc.scalar.copy(out=trk[:], in_=ptr[:])
                        trs.append(trk)
                    po = popool.tile([PB, out_c], f32)
                    for k in range(NK):
                        nc.tensor.matmul(
                            out=po[:],
                            lhsT=trs[k][:],
                            rhs=w_sb[:, k * out_c : (k + 1) * out_c],
                            start=(k == 0),
                            stop=(k == NK - 1),
                        )
                    osb = opool.tile([PB, out_c], f32, tag="o")
                    nc.vector.tensor_tensor(
                        out=osb[:], in0=po[:], in1=bias_sb[:],
                        op=mybir.AluOpType.add,
                    )
                    nc.sync.dma_start(
                        out=out_d[b, cur_s * PB : (cur_s + 1) * PB, :],
                        in_=osb[:],
                    )

    with TileContext(nc) as tc:
        with (
            tc.tile_pool(name="const", bufs=1) as cpool,
            tc.tile_pool(name="gl", bufs=3) as gl_pool,
            tc.tile_pool(name="gh", bufs=3) as gh_pool,
            tc.tile_pool(name="s", bufs=6) as spool,
            tc.tile_pool(name="seg", bufs=2) as segpool,
            tc.tile_pool(name="tr", bufs=4) as trpool,
            tc.tile_pool(name="o", bufs=4) as opool,
            tc.tile_pool(name="bias", bufs=2) as bpool,
            tc.tile_pool(name="pacc", bufs=2, space="PSUM") as papool,
            tc.tile_pool(name="ptr", bufs=2, space="PSUM") as ptpool,
            tc.tile_pool(name="pout", bufs=2, space="PSUM") as popool,
        ):
            pools = (cpool, gl_pool, gh_pool, spool, segpool, trpool, opool,
                     bpool, papool, ptpool, popool)
            if reps == 1:
                body(nc, tc, pools)
            else:
                with tc.For_i(0, reps, 1):
                    body(nc, tc, pools)

    nc.compile()
    return nc


def _host_arrays(x, weight):
    xi = np.ascontiguousarray(np.concatenate([x[b] for b in range(B)], axis=1))
    NK = IN_C // PB
    wT = np.ascontiguousarray(
        np.concatenate([weight[k * PB : (k + 1) * PB] for k in range(NK)], axis=1)
    )
    iota2d = np.ascontiguousarray(np.tile(np.arange(PB, dtype=np.float32), (PB, 1)))
    ident = np.eye(PB, dtype=np.float32)
    return xi, wT, iota2d, ident


def _in_maps(rows, cols, vals, weight, bias, x, plan):
    NB, TPBL, TPBH, rowsets = plan
    xi, wT, iota2d, ident = _host_arrays(x, weight)
    if GDT == "f16":
        xi = xi.astype(np.float16)
    elif GDT == "f8":
        import ml_dtypes

        xi = xi.astype(ml_dtypes.float8_e3m4)
    order_r = np.argsort(rows, kind="stable")
    bnd_r = np.searchsorted(rows[order_r], np.arange(N_OUT + 1))
    maps = []
    for c in range(N_CORES):
        rowsT, valsT, idxLW, idxHW, bias_c = _pack_core(
            c, plan, rows, cols, vals, bias, order_r, bnd_r
        )
        if idxLW.size == 0:
            idxLW = np.zeros((PB, 8), np.int16)
        if idxHW.size == 0:
            idxHW = np.zeros((PB, 8), np.int16)
        maps.append(
            {
                "xi": xi,
                "idxLW": idxLW,
                "idxHW": idxHW,
                "rowsT": rowsT,
                "valsT": valsT,
                "biasC": bias_c,
                "wT": wT,
                "iota": iota2d,
                "ident": ident,
            }
        )
    return maps


_DEV_CACHE: dict = {}


def _make_runner(nc, in_maps):
    """Compile nc via PJRT and return run(trials)->list[sec] with the big
    input arrays kept resident on device (cached by content fingerprint), so
    per-call wall time is dispatch + small zero-output upload + execution."""
    import jax
    from jax.experimental.shard_map import shard_map
    from jax.sharding import Mesh, NamedSharding, PartitionSpec

    import concourse.mybir as mybir
    from concourse.bass2jax import (
        _bass_exec_p,
        install_neuronx_cc_hook,
        partition_id_tensor,
    )

    install_neuronx_cc_hook()
    partition_name = nc.partition_id_tensor.name if nc.partition_id_tensor else None
    in_names, out_names, out_avals, zero_outs = [], [], [], []
    for alloc in nc.m.functions[0].allocations:
        if not isinstance(alloc, mybir.MemoryLocationSet):
            continue
        name = alloc.memorylocations[0].name
        if alloc.kind == "ExternalInput":
            if name != partition_name:
                in_names.append(name)
        elif alloc.kind == "ExternalOutput":
            shape = tuple(alloc.tensor_shape)
            dtype = mybir.dt.np(alloc.dtype)
            out_names.append(name)
            out_avals.append(jax.core.ShapedArray(shape, dtype))
            zero_outs.append(np.zeros(shape, dtype))
    n_params = len(in_names)
    n_outs = len(out_avals)
    all_in_names = list(in_names) + list(out_names)
    if partition_name is not None:
        all_in_names.append(partition_name)
    donate = tuple(range(n_params, n_params + n_outs))

    def _body(*args):
        operands = list(args)
        if partition_name is not None:
            operands.append(partition_id_tensor())
        return tuple(
            _bass_exec_p.bind(
                *operands,
                out_avals=tuple(out_avals),
                in_names=tuple(all_in_names),
                out_names=tuple(out_names),
                lowering_input_output_aliases=(),
                sim_require_finite=True,
                sim_require_nnan=True,
                nc=nc,
            )
        )

    devices = jax.devices()[:N_CORES]
    mesh = Mesh(np.asarray(devices), ("core",))
    sharded = jax.jit(
        shard_map(
            _body,
            mesh=mesh,
            in_specs=(PartitionSpec("core"),) * (n_params + n_outs),
            out_specs=(PartitionSpec("core"),) * len(out_names),
            check_rep=False,
        ),
        donate_argnums=donate,
        keep_unused=True,
    )

    sh = NamedSharding(mesh, PartitionSpec("core"))
    dev_in = []
    for name in in_names:
        arrs = [np.asarray(m[name]) for m in in_maps]
        a0 = arrs[0]
        step = max(1, a0.size // 64)
        fp = (a0.shape, str(a0.dtype), len(arrs),
              a0.reshape(-1)[::step][:64].tobytes())
        hit = _DEV_CACHE.get(name)
        if hit is not None and hit[0] == fp:
            dev_in.append(hit[1])
            continue
        darr = jax.device_put(np.concatenate(arrs, axis=0), sh)
        darr.block_until_ready()
        _DEV_CACHE[name] = (fp, darr)
        dev_in.append(darr)

    def run(trials=6, warm=1):
        import time as _time

        ts = []
        for t in range(warm + trials):
            zeros = [
                jax.device_put(
                    np.zeros((N_CORES * z.shape[0], *z.shape[1:]), z.dtype), sh
                )
                for z in zero_outs
            ]
            for z in zeros:
                z.block_until_ready()
            t0 = _time.perf_counter()
            outs = sharded(*dev_in, *zeros)
            for o in outs:
                o.block_until_ready()
            dt = _time.perf_counter() - t0
            if t >= warm:
                ts.append(dt)
        return ts

    return run


def time_hw(inputs, reps=(1, 129), trials=6):
    """HW ns/iter via wall-clock delta between For_i repeat-count variants.
    Inputs stay resident on device across calls, so the delta isolates pure
    device execution time; upload/dispatch costs are identical and cancel."""
    rows = np.asarray(inputs["rows"], dtype=np.int64)
    cols = np.asarray(inputs["cols"], dtype=np.int64)
    vals = np.asarray(inputs["vals"], dtype=np.float32)
    x = np.asarray(inputs["x"], dtype=np.float32)
    weight = np.asarray(inputs["weight"], dtype=np.float32)
    bias = np.asarray(inputs["bias"], dtype=np.float32)

    plan = _plan(rows, cols)
    NB, TPBL, TPBH, rowsets = plan
    maps = _in_maps(rows, cols, vals, weight, bias, x, plan)

    best = {}
    for r in reps:
        nc = _build(NB, TPBL, TPBH, N_IN, B, IN_C, OUT_C, GCH, reps=r)
        run = _make_runner(nc, maps)
        ts = run(trials=trials)
        best[r] = min(ts)
        print(f"reps={r}: calls {[f'{t*1e3:.1f}ms' for t in ts]}")
    r1, r2 = min(reps), max(reps)
    ns = (best[r2] - best[r1]) / (r2 - r1) * 1e9
    return ns


# ---------------------------------------------------------------- entry point
def kernel(x, rows, cols, vals, weight, bias):
    global LAST_RESULTS
    from concourse.bass_utils import run_bass_kernel_spmd

    x = np.asarray(x, dtype=np.float32)
    rows = np.asarray(rows, dtype=np.int64)
    cols = np.asarray(cols, dtype=np.int64)
    vals = np.asarray(vals, dtype=np.float32)
    weight = np.asarray(weight, dtype=np.float32)
    bias = np.asarray(bias, dtype=np.float32)

    plan = _plan(rows, cols)
    NB, TPBL, TPBH, rowsets = plan

    key = (NB, tuple(TPBL), tuple(TPBH), GCH)
    if key not in _CACHE:
        _CACHE.clear()
        _CACHE[key] = _build(NB, TPBL, TPBH, N_IN, B, IN_C, OUT_C, GCH)
    nc = _CACHE[key]

    maps = _in_maps(rows, cols, vals, weight, bias, x, plan)
    res = run_bass_kernel_spmd(nc, maps, core_ids=list(range(N_CORES)))
    LAST_RESULTS = res

    out = np.empty((B, N_OUT, OUT_C), dtype=np.float32)
    for c in range(N_CORES):
        oc = res.results[c]["out"]
        for s in range(NB):
            rowlist = rowsets[c][s]
            if rowlist is None or len(rowlist) == 0:
                continue
            out[:, rowlist, :] = oc[:, s * PB : s * PB + len(rowlist), :]
    return out



# revision 7
# speedup vs baseline: 1.9219x; 1.6931x over previous
"""Trainium2 Bass kernel for batched GNN message passing.

Computes, for each batch b:
    ax[b] = segment_sum(x[b][cols] * vals[:, None], rows, num_segments=N_OUT)
    out[b] = ax[b] @ weight + bias

Strategy (8 NeuronCores):
  * The two batches share one graph, so x is interleaved host-side into
    xi[n] = concat(x[0,n], x[1,n]) -> [N_IN, 2*IN_C]; one gathered row
    serves both batches (halves descriptor count, shares the selection
    matrix and the PE weight load between batches).
  * Output rows are split into 128-row blocks. Blocks are dealt to the 8
    cores sorted by edge count (rounds of 8 similar-sized blocks) so every
    core runs an identical program: NB block-slots, slot s processing
    TPBL[s] + TPBH[s] tiles of 128 edges (dma_gather indices are int16,
    so edges are split into col < 32768 gathered from xi[0:] and
    col >= 32768 gathered from xi[32768:]).
  * Per 128-edge tile: dma_gather of 128 rows (2KB each) from HBM, build
    the scaled selection matrix S^T[e, r] = vals[e] * (rloc[e] == r) with
    one fused DVE op, then PE matmul psum[128 rows, 2*IN_C] += S^T.T @ G
    accumulated over the slot's tiles (the segment sum).
  * Per slot epilogue: transpose the segment sum (PE), project with
    weight (PE, K=256 in two 128-chunks), add bias (DVE) and DMA out.
"""

import math

import numpy as np

# ---------------------------------------------------------------- problem dims
B = 2
N_IN = 50000
N_OUT = 12500
NNZ = 500000
IN_C = 256
OUT_C = 256
N_CORES = 8
PB = 128  # rows per output block == partition count
H16 = 32768  # int16 index limit for dma_gather

import os as _os

GCH = int(_os.environ.get("K_GCH", "8"))  # 128-edge tiles per dma_gather call
# gather dtype: "f32" | "f32r" (tf32-like) | "f16" | "f8" (e3m4)
GDT = _os.environ.get("K_GDT", "f32r")
# SWDGE queues: gather descriptor-gen spread over NSWQ Q7 core pairs
NSWQ = int(_os.environ.get("K_NSWQ", "4"))
# timing decomposition: "" (full) | "gather" (skip gathers) | "compute"
# (skip DVE/PE/epilogue) — timing-only variants, numerics are garbage
K_SKIP = _os.environ.get("K_SKIP", "")

_CACHE = {}
LAST_RESULTS = None


# ---------------------------------------------------------------- host planning
def _plan(rows, cols):
    """Pack output rows into (core, slot) bins of <=128 rows, balancing the
    low/high edge loads so every slot's tile counts are tight and uniform.
    Slot s everywhere holds TPBL[s] low tiles + TPBH[s] high tiles."""
    Lr = np.bincount(rows[cols < H16], minlength=N_OUT)
    Hr = np.bincount(rows[cols >= H16], minlength=N_OUT)
    NB = -(-N_OUT // (PB * N_CORES))
    nbins = N_CORES * NB
    tL = max(Lr.sum() / nbins, 1.0)
    tH = max(Hr.sum() / nbins, 1.0)

    order = np.argsort(-(Lr + Hr), kind="stable")
    binL = np.zeros(nbins)
    binH = np.zeros(nbins)
    binN = np.zeros(nbins, dtype=np.int64)
    bin_rows = [[] for _ in range(nbins)]
    for r in order:
        score = np.maximum((binL + Lr[r]) / tL, (binH + Hr[r]) / tH)
        score[binN >= PB] = np.inf
        b = int(score.argmin())
        binL[b] += Lr[r]
        binH[b] += Hr[r]
        binN[b] += 1
        bin_rows[b].append(int(r))

    # group bins into slots by (L-quantum, H) so per-slot maxima stay tight
    q = np.lexsort((-binH, -(-(-binL.astype(np.int64) // PB))))
    rowsets = [[None] * NB for _ in range(N_CORES)]
    TPBL, TPBH = [], []
    for s in range(NB):
        grp = q[s * N_CORES : (s + 1) * N_CORES]
        tl = int(-(-int(binL[grp].max()) // PB))
        th = int(-(-int(binH[grp].max()) // PB))
        if tl + th == 0:
            tl = 1
        TPBL.append(tl)
        TPBH.append(th)
        for c, b in enumerate(grp):
            rowsets[c][s] = np.array(sorted(bin_rows[b]), dtype=np.int64)
    return NB, TPBL, TPBH, rowsets


def _wrap16(flat):
    """int16 index stream -> dma_gather layout: idx i at partition i%16,
    col i//16, replicated across the 8 gpsimd core groups."""
    n = len(flat)
    assert n % 16 == 0
    w = np.ascontiguousarray(flat.reshape(n // 16, 16).T.astype(np.int16))
    return np.ascontiguousarray(np.tile(w, (8, 1)))


def _pack_core(c, plan, rows, cols, vals, bias, order_r, bnd_r):
    """Per-core arrays: rowsT/valsT [128, NT] (col j = tile j, partition p
    = edge j*128+p), wrapped int16 gather index streams, bias."""
    NB, TPBL, TPBH, rowsets = plan
    NT = sum(TPBL) + sum(TPBH)
    rloc_flat = np.zeros(NT * PB, dtype=np.float32)
    vals_flat = np.zeros(NT * PB, dtype=np.float32)
    lowE, highE = [], []
    bias_c = np.zeros((NB * PB, OUT_C), dtype=np.float32)
    pos = 0
    for s in range(NB):
        rowlist = rowsets[c][s]
        nr = len(rowlist)
        if nr:
            per_row = [order_r[bnd_r[r] : bnd_r[r + 1]] for r in rowlist]
            eids = (np.concatenate(per_row) if per_row
                    else np.empty(0, np.int64))
            rloc = np.repeat(np.arange(nr), [len(e) for e in per_row])
            m = cols[eids] < H16
            lo, lo_rl = eids[m], rloc[m]
            hi, hi_rl = eids[~m], rloc[~m]
            # column-sorted gather streams sweep HBM monotonically
            ol = np.argsort(cols[lo], kind="stable")
            lo, lo_rl = lo[ol], lo_rl[ol]
            oh = np.argsort(cols[hi], kind="stable")
            hi, hi_rl = hi[oh], hi_rl[oh]
            bias_c[s * PB : s * PB + nr] = bias[rowlist]
        else:
            lo = hi = np.empty(0, dtype=np.int64)
            lo_rl = hi_rl = np.empty(0, dtype=np.int64)
        for lst, rl, tpb, base, acc in (
            (lo, lo_rl, TPBL[s], 0, lowE),
            (hi, hi_rl, TPBH[s], H16, highE),
        ):
            k = tpb * PB
            if k == 0:
                assert len(lst) == 0
                continue
            ne = len(lst)
            assert ne <= k, (ne, k)
            rloc_flat[pos : pos + ne] = rl
            vals_flat[pos : pos + ne] = vals[lst]
            cc = np.zeros(k, dtype=np.int64)
            cc[:ne] = cols[lst] - base
            acc.append(cc)
            pos += k
    assert pos == NT * PB

    rowsT = np.ascontiguousarray(rloc_flat.reshape(NT, PB).T)
    valsT = np.ascontiguousarray(vals_flat.reshape(NT, PB).T)
    idxLW = _wrap16(np.concatenate(lowE) if lowE else np.empty(0, np.int64))
    idxHW = _wrap16(np.concatenate(highE) if highE else np.empty(0, np.int64))
    return rowsT, valsT, idxLW, idxHW, bias_c


# ---------------------------------------------------------------- device build
def _build(NB, TPBL, TPBH, n_in, nbatch, in_c, out_c, gch, reps=1, gdt=None):
    import concourse.bacc as bacc
    import concourse.mybir as mybir
    from concourse.tile import TileContext

    f32 = mybir.dt.float32
    i16 = mybir.dt.int16
    gdt = {"f32": f32, "f32r": mybir.dt.float32r,
           "f16": mybir.dt.float16, "f8": mybir.dt.float8e3}[gdt or GDT]
    C2 = nbatch * in_c  # gathered row width (both batches)
    NL, NH = sum(TPBL), sum(TPBH)
    NT = NL + NH
    NK = in_c // PB  # K-chunks in the weight projection

    nc = bacc.Bacc("TRN2", target_bir_lowering=False, debug=False,
                   num_devices=N_CORES, num_swdge_queues=NSWQ)

    xi_d = nc.dram_tensor("xi", [n_in, C2], gdt, kind="ExternalInput")
    idxl_d = nc.dram_tensor("idxLW", [PB, max(NL * 8, 8)], i16, kind="ExternalInput")
    idxh_d = nc.dram_tensor("idxHW", [PB, max(NH * 8, 8)], i16, kind="ExternalInput")
    rows_d = nc.dram_tensor("rowsT", [PB, NT], f32, kind="ExternalInput")
    vals_d = nc.dram_tensor("valsT", [PB, NT], f32, kind="ExternalInput")
    bias_d = nc.dram_tensor("biasC", [NB * PB, out_c], f32, kind="ExternalInput")
    w_d = nc.dram_tensor("wT", [PB, NK * out_c], f32, kind="ExternalInput")
    iota_d = nc.dram_tensor("iota", [PB, PB], f32, kind="ExternalInput")
    ident_d = nc.dram_tensor("ident", [PB, PB], f32, kind="ExternalInput")
    out_d = nc.dram_tensor("out", [nbatch, NB * PB, out_c], f32,
                           kind="ExternalOutput")

    # processing order: per slot, its low tiles then its high tiles.
    # each entry: (slot, first, last, stream ('l'/'h'), stream tile index)
    sched = []
    nl = nh = 0
    for s in range(NB):
        tpb = TPBL[s] + TPBH[s]
        t = 0
        for _ in range(TPBL[s]):
            sched.append((s, t == 0, t == tpb - 1, "l", nl))
            nl += 1
            t += 1
        for _ in range(TPBH[s]):
            sched.append((s, t == 0, t == tpb - 1, "h", nh))
            nh += 1
            t += 1

    def body(nc, tc, pools):
        (cpool, gl_pool, gh_pool, spool, segpool, trpool, opool, bpool,
         papool, ptpool, popool) = pools
        iota_sb = cpool.tile([PB, PB], f32, tag="iota")
        ident_sb = cpool.tile([PB, PB], f32, tag="ident")
        w_sb = cpool.tile([PB, NK * out_c], f32, tag="w")
        idxl_sb = cpool.tile([PB, max(NL * 8, 8)], i16, tag="idxl")
        idxh_sb = cpool.tile([PB, max(NH * 8, 8)], i16, tag="idxh")
        rows_sb = cpool.tile([PB, NT], f32, tag="rows")
        vals_sb = cpool.tile([PB, NT], f32, tag="vals")
        nc.sync.dma_start(out=iota_sb[:], in_=iota_d[:])
        nc.sync.dma_start(out=ident_sb[:], in_=ident_d[:])
        nc.sync.dma_start(out=w_sb[:], in_=w_d[:])
        nc.sync.dma_start(out=idxl_sb[:], in_=idxl_d[:])
        nc.sync.dma_start(out=idxh_sb[:], in_=idxh_d[:])
        nc.sync.dma_start(out=rows_sb[:], in_=rows_d[:])
        nc.sync.dma_start(out=vals_sb[:], in_=vals_d[:])

        qctr = [0]

        def gather(pool, tag, idx_sb, src_ap, t0, nstream):
            n = min(gch, nstream - t0)
            g = pool.tile([PB, gch * C2], gdt, tag=tag)
            nc.gpsimd.dma_gather(
                out_ap=g[:, : n * C2].rearrange("p (t e) -> p t e", e=C2),
                in_ap=src_ap,
                idxs_ap=idx_sb[:, t0 * 8 : (t0 + n) * 8],
                num_idxs=n * PB,
                num_idxs_reg=n * PB,
                elem_size=C2,
                queue_num=qctr[0] % NSWQ,
            )
            qctr[0] += 1
            return g

        skip_gather = K_SKIP == "gather"
        skip_compute = K_SKIP == "compute"
        gconst = None
        if skip_gather:
            gconst = cpool.tile([PB, gch * C2], gdt, tag="gconst")
            nc.gpsimd.dma_gather(
                out_ap=gconst[:].rearrange("p (t e) -> p t e", e=C2),
                in_ap=xi_d[:],
                idxs_ap=idxl_sb[:, 0:8],
                num_idxs=PB,
                num_idxs_reg=PB,
                elem_size=C2,
                queue_num=0,
            )

        gl = gh = None
        pacc = None
        bias_sb = None
        cur_s = -1
        for j in range(NT):
            s, first, last, stream, st = sched[j]
            if stream == "l":
                if not skip_gather and st % gch == 0:
                    gl = gather(gl_pool, "gl", idxl_sb, xi_d[:], st, NL)
                g, off = (gconst, 0) if skip_gather else (gl, st % gch)
            else:
                if not skip_gather and st % gch == 0:
                    gh = gather(gh_pool, "gh", idxh_sb, xi_d[H16:, :], st, NH)
                g, off = (gconst, 0) if skip_gather else (gh, st % gch)
            if skip_compute:
                continue
            if first:
                cur_s = s
                bias_sb = bpool.tile([PB, out_c], f32, tag="bias")
                nc.sync.dma_start(
                    out=bias_sb[:], in_=bias_d[s * PB : (s + 1) * PB, :]
                )
                pacc = papool.tile([PB, C2], f32)

            s_t = spool.tile([PB, PB], gdt, tag="s")
            nc.vector.tensor_scalar(
                out=s_t[:],
                in0=iota_sb[:],
                scalar1=rows_sb[:, j : j + 1],
                scalar2=vals_sb[:, j : j + 1],
                op0=mybir.AluOpType.is_equal,
                op1=mybir.AluOpType.mult,
            )
            nc.tensor.matmul(
                out=pacc[:],
                lhsT=s_t[:],
                rhs=g[:, off * C2 : (off + 1) * C2],
                start=first,
                stop=last,
            )

            if last:
                seg = segpool.tile([PB, C2], f32, tag="seg")
                nc.scalar.copy(out=seg[:], in_=pacc[:])
                for b in range(nbatch):
                    trs = []
                    for k in range(NK):
                        ptr = ptpool.tile([PB, PB], f32)
                        nc.tensor.transpose(
                            out=ptr[:],
                            in_=seg[:, b * in_c + k * PB : b * in_c + (k + 1) * PB],
                            identity=ident_sb[:],
                        )
                        trk = trpool.tile([PB, PB], f32, tag="tr")
                        nc.scalar.copy(out=trk[:], in_=ptr[:])
                        trs.append(trk)
                    po = popool.tile([PB, out_c], f32)
                    for k in range(NK):
                        nc.tensor.matmul(
                            out=po[:],
                            lhsT=trs[k][:],
                            rhs=w_sb[:, k * out_c : (k + 1) * out_c],
                            start=(k == 0),
                            stop=(k == NK - 1),
                        )
                    osb = opool.tile([PB, out_c], f32, tag="o")
                    nc.vector.tensor_tensor(
                        out=osb[:], in0=po[:], in1=bias_sb[:],
                        op=mybir.AluOpType.add,
                    )
                    nc.sync.dma_start(
                        out=out_d[b, cur_s * PB : (cur_s + 1) * PB, :],
                        in_=osb[:],
                    )

    with TileContext(nc) as tc:
        with (
            tc.tile_pool(name="const", bufs=1) as cpool,
            tc.tile_pool(name="gl", bufs=3) as gl_pool,
            tc.tile_pool(name="gh", bufs=3) as gh_pool,
            tc.tile_pool(name="s", bufs=6) as spool,
            tc.tile_pool(name="seg", bufs=2) as segpool,
            tc.tile_pool(name="tr", bufs=4) as trpool,
            tc.tile_pool(name="o", bufs=4) as opool,
            tc.tile_pool(name="bias", bufs=2) as bpool,
            tc.tile_pool(name="pacc", bufs=2, space="PSUM") as papool,
            tc.tile_pool(name="ptr", bufs=2, space="PSUM") as ptpool,
            tc.tile_pool(name="pout", bufs=2, space="PSUM") as popool,
        ):
            pools = (cpool, gl_pool, gh_pool, spool, segpool, trpool, opool,
                     bpool, papool, ptpool, popool)
            if reps == 1:
                body(nc, tc, pools)
            else:
                with tc.For_i(0, reps, 1):
                    body(nc, tc, pools)

    nc.compile()
    return nc


def _host_arrays(x, weight):
    xi = np.ascontiguousarray(np.concatenate([x[b] for b in range(B)], axis=1))
    NK = IN_C // PB
    wT = np.ascontiguousarray(
        np.concatenate([weight[k * PB : (k + 1) * PB] for k in range(NK)], axis=1)
    )
    iota2d = np.ascontiguousarray(np.tile(np.arange(PB, dtype=np.float32), (PB, 1)))
    ident = np.eye(PB, dtype=np.float32)
    return xi, wT, iota2d, ident


def _in_maps(rows, cols, vals, weight, bias, x, plan):
    NB, TPBL, TPBH, rowsets = plan
    xi, wT, iota2d, ident = _host_arrays(x, weight)
    if GDT == "f16":
        xi = xi.astype(np.float16)
    elif GDT == "f8":
        import ml_dtypes

        xi = xi.astype(ml_dtypes.float8_e3m4)
    order_r = np.argsort(rows, kind="stable")
    bnd_r = np.searchsorted(rows[order_r], np.arange(N_OUT + 1))
    maps = []
    for c in range(N_CORES):
        rowsT, valsT, idxLW, idxHW, bias_c = _pack_core(
            c, plan, rows, cols, vals, bias, order_r, bnd_r
        )
        if idxLW.size == 0:
            idxLW = np.zeros((PB, 8), np.int16)
        if idxHW.size == 0:
            idxHW = np.zeros((PB, 8), np.int16)
        maps.append(
            {
                "xi": xi,
                "idxLW": idxLW,
                "idxHW": idxHW,
                "rowsT": rowsT,
                "valsT": valsT,
                "biasC": bias_c,
                "wT": wT,
                "iota": iota2d,
                "ident": ident,
            }
        )
    return maps


_DEV_CACHE: dict = {}


def _make_runner(nc, in_maps):
    """Compile nc via PJRT and return run(trials)->list[sec] with the big
    input arrays kept resident on device (cached by content fingerprint), so
    per-call wall time is dispatch + small zero-output upload + execution."""
    import jax
    from jax.experimental.shard_map import shard_map
    from jax.sharding import Mesh, NamedSharding, PartitionSpec

    import concourse.mybir as mybir
    from concourse.bass2jax import (
        _bass_exec_p,
        install_neuronx_cc_hook,
        partition_id_tensor,
    )

    install_neuronx_cc_hook()
    partition_name = nc.partition_id_tensor.name if nc.partition_id_tensor else None
    in_names, out_names, out_avals, zero_outs = [], [], [], []
    for alloc in nc.m.functions[0].allocations:
        if not isinstance(alloc, mybir.MemoryLocationSet):
            continue
        name = alloc.memorylocations[0].name
        if alloc.kind == "ExternalInput":
            if name != partition_name:
                in_names.append(name)
        elif alloc.kind == "ExternalOutput":
            shape = tuple(alloc.tensor_shape)
            dtype = mybir.dt.np(alloc.dtype)
            out_names.append(name)
            out_avals.append(jax.core.ShapedArray(shape, dtype))
            zero_outs.append(np.zeros(shape, dtype))
    n_params = len(in_names)
    n_outs = len(out_avals)
    all_in_names = list(in_names) + list(out_names)
    if partition_name is not None:
        all_in_names.append(partition_name)
    donate = tuple(range(n_params, n_params + n_outs))

    def _body(*args):
        operands = list(args)
        if partition_name is not None:
            operands.append(partition_id_tensor())
        return tuple(
            _bass_exec_p.bind(
                *operands,
                out_avals=tuple(out_avals),
                in_names=tuple(all_in_names),
                out_names=tuple(out_names),
                lowering_input_output_aliases=(),
                sim_require_finite=True,
                sim_require_nnan=True,
                nc=nc,
            )
        )

    devices = jax.devices()[:N_CORES]
    mesh = Mesh(np.asarray(devices), ("core",))
    sharded = jax.jit(
        shard_map(
            _body,
            mesh=mesh,
            in_specs=(PartitionSpec("core"),) * (n_params + n_outs),
            out_specs=(PartitionSpec("core"),) * len(out_names),
            check_rep=False,
        ),
        donate_argnums=donate,
        keep_unused=True,
    )

    sh = NamedSharding(mesh, PartitionSpec("core"))
    dev_in = []
    for name in in_names:
        arrs = [np.asarray(m[name]) for m in in_maps]
        a0 = arrs[0]
        step = max(1, a0.size // 64)
        fp = (a0.shape, str(a0.dtype), len(arrs),
              a0.reshape(-1)[::step][:64].tobytes())
        hit = _DEV_CACHE.get(name)
        if hit is not None and hit[0] == fp:
            dev_in.append(hit[1])
            continue
        darr = jax.device_put(np.concatenate(arrs, axis=0), sh)
        darr.block_until_ready()
        _DEV_CACHE[name] = (fp, darr)
        dev_in.append(darr)

    def run(trials=6, warm=1):
        import time as _time

        ts = []
        for t in range(warm + trials):
            zeros = [
                jax.device_put(
                    np.zeros((N_CORES * z.shape[0], *z.shape[1:]), z.dtype), sh
                )
                for z in zero_outs
            ]
            for z in zeros:
                z.block_until_ready()
            t0 = _time.perf_counter()
            outs = sharded(*dev_in, *zeros)
            for o in outs:
                o.block_until_ready()
            dt = _time.perf_counter() - t0
            if t >= warm:
                ts.append(dt)
        return ts

    return run


def time_hw(inputs, reps=(1, 129), trials=6):
    """HW ns/iter via wall-clock delta between For_i repeat-count variants.
    Inputs stay resident on device across calls, so the delta isolates pure
    device execution time; upload/dispatch costs are identical and cancel."""
    rows = np.asarray(inputs["rows"], dtype=np.int64)
    cols = np.asarray(inputs["cols"], dtype=np.int64)
    vals = np.asarray(inputs["vals"], dtype=np.float32)
    x = np.asarray(inputs["x"], dtype=np.float32)
    weight = np.asarray(inputs["weight"], dtype=np.float32)
    bias = np.asarray(inputs["bias"], dtype=np.float32)

    plan = _plan(rows, cols)
    NB, TPBL, TPBH, rowsets = plan
    maps = _in_maps(rows, cols, vals, weight, bias, x, plan)

    best = {}
    for r in reps:
        nc = _build(NB, TPBL, TPBH, N_IN, B, IN_C, OUT_C, GCH, reps=r)
        run = _make_runner(nc, maps)
        ts = run(trials=trials)
        best[r] = min(ts)
        print(f"reps={r}: calls {[f'{t*1e3:.1f}ms' for t in ts]}")
    r1, r2 = min(reps), max(reps)
    ns = (best[r2] - best[r1]) / (r2 - r1) * 1e9
    return ns


# ---------------------------------------------------------------- entry point
def kernel(x, rows, cols, vals, weight, bias):
    global LAST_RESULTS
    from concourse.bass_utils import run_bass_kernel_spmd

    x = np.asarray(x, dtype=np.float32)
    rows = np.asarray(rows, dtype=np.int64)
    cols = np.asarray(cols, dtype=np.int64)
    vals = np.asarray(vals, dtype=np.float32)
    weight = np.asarray(weight, dtype=np.float32)
    bias = np.asarray(bias, dtype=np.float32)

    plan = _plan(rows, cols)
    NB, TPBL, TPBH, rowsets = plan

    key = (NB, tuple(TPBL), tuple(TPBH), GCH)
    if key not in _CACHE:
        _CACHE.clear()
        _CACHE[key] = _build(NB, TPBL, TPBH, N_IN, B, IN_C, OUT_C, GCH)
    nc = _CACHE[key]

    maps = _in_maps(rows, cols, vals, weight, bias, x, plan)
    res = run_bass_kernel_spmd(nc, maps, core_ids=list(range(N_CORES)))
    LAST_RESULTS = res

    out = np.empty((B, N_OUT, OUT_C), dtype=np.float32)
    for c in range(N_CORES):
        oc = res.results[c]["out"]
        for s in range(NB):
            rowlist = rowsets[c][s]
            if rowlist is None or len(rowlist) == 0:
                continue
            out[:, rowlist, :] = oc[:, s * PB : s * PB + len(rowlist), :]
    return out

